# revision 1
# baseline (speedup 1.0000x reference)
"""GatedDeltaNet on Trainium2, 8 NeuronCores, chunked-parallel delta rule.

Sharding: 24 streams = (batch 2) x (head 6) x (dv-half 2); 3 streams/core.
Core c (c<4 -> b=0, else b=1, cb = c%4) owns:
  stream 0: (h=cb,   j=0)   "pair A" = head cb
  stream 1: (h=cb,   j=1)
  stream 2: (h=4+cb//2, j=cb%2)   "pair B"
Launch A (per stream): projections + causal conv + silu + l2norm + chunked
gated-delta recurrence (chunk C=128, UT-inverse via K-term Horner series),
emitting o[stream] = [T, 256] bf16.
Launch B (token-sharded 8x1024): per-head RMSNorm over full DV, SiLU output
gate, and the Wo projection.
"""
import numpy as np
import ml_dtypes

H, DK, DV = 6, 256, 512
HID = 2048
CONV = 4
NORM_EPS = 1e-5
N_CORES = 8
B, T = 2, 4096
C = 128                  # chunk length
TB = 512                 # token block in launch A
NBLK = T // TB           # 8
NCH = TB // C            # 4 chunks per block
KT = HID // 128          # 16 contraction tiles
K_HORNER = 4
BF = ml_dtypes.bfloat16

LAST_HW_EXEC_NS = None


def _bf16(a):
    """fp32 ndarray -> bfloat16 (round-to-nearest-even via uint trick)."""
    a = np.ascontiguousarray(a, np.float32)
    u = a.view(np.uint32)
    r = ((u >> 16) & 1) + 0x7FFF
    return ((u + r) >> 16).astype(np.uint16).view(BF)




def _split_multi_waits(nc):
    """Walrus under this toolchain rejects instructions with >1 semaphore
    wait ("Too many sync wait commands").  Split: insert same-engine wait-only
    NoOps before any instruction carrying multiple waits."""
    import concourse.mybir as mybir
    n_split = 0
    for f in nc.m.functions:
        for bb in f.blocks:
            new_insts = []
            for inst in bb.instructions:
                si = getattr(inst, 'sync_info', None)
                if si is not None and si.on_wait and len(si.on_wait) > 1:
                    waits = list(si.on_wait)
                    for w in waits[:-1]:
                        nop = mybir.InstNoOp(
                            name=f"{inst.name}-wsplit{n_split}",
                            engine=inst.engine,
                            ins=[], outs=[],
                            sync_info=mybir.SyncInfo(on_wait=[w], on_update=[]),
                        )
                        n_split += 1
                        new_insts.append(nop)
                    inst.sync_info = mybir.SyncInfo(
                        on_wait=[waits[-1]], on_update=list(si.on_update or []))
                new_insts.append(inst)
            bb.instructions[:] = new_insts
    return n_split


# ---------------------------------------------------------------------------
# Launch A
# ---------------------------------------------------------------------------

def build_launch_a(T_total=T, k_horner=K_HORNER):
    import concourse.bass as bass
    import concourse.mybir as mybir
    import concourse.tile as tile
    from concourse.masks import make_identity

    fp32 = mybir.dt.float32
    bf16 = mybir.dt.bfloat16
    AF = mybir.ActivationFunctionType
    OP = mybir.AluOpType

    nblk = T_total // TB
    nc = bass.Bass()
    xT = nc.declare_dram_parameter("xT", [HID, T_total], bf16, isOutput=False)
    wqk = nc.declare_dram_parameter("wqk", [HID, 2 * 512], bf16, isOutput=False)
    wv = nc.declare_dram_parameter("wv", [HID, 3 * 256], bf16, isOutput=False)
    wba = nc.declare_dram_parameter("wba", [HID, 2 * 33], bf16, isOutput=False)
    cw = nc.declare_dram_parameter("cw", [128, 14, CONV], fp32, isOutput=False)
    scal = nc.declare_dram_parameter("scal", [128, 4], fp32, isOutput=False)
    o_out = nc.declare_dram_parameter("o", [3, T_total, 256], bf16, isOutput=True)

    xT_r = xT.rearrange("(kt p) t -> p kt t", p=128)
    wqk_r = wqk.rearrange("(kt p) n -> p kt n", p=128)
    wv_r = wv.rearrange("(kt p) n -> p kt n", p=128)
    wba_r = wba.rearrange("(kt p) n -> p kt n", p=128)

    # conv-weight instance index: [qA0,qA1,kA0,kA1,qB0,qB1,kB0,kB1] then v:
    # [vs0_0, vs0_1, vs1_0, vs1_1, vs2_0, vs2_1]
    def cwi_qk(pair, proj, d):
        return pair * 4 + proj * 2 + d

    def cwi_v(stream, d):
        return 8 + stream * 2 + d

    with tile.TileContext(nc) as tc:
        import contextlib
        ctx = contextlib.ExitStack()
        with ctx:
            singles = ctx.enter_context(tc.tile_pool(name="singles", bufs=1))
            wpool = ctx.enter_context(tc.tile_pool(name="wpool", bufs=1))
            xpool = ctx.enter_context(tc.tile_pool(name="xpool", bufs=2))
            stgp = ctx.enter_context(tc.tile_pool(name="stgp", bufs=3))
            kapp = ctx.enter_context(tc.tile_pool(name="kapp", bufs=2))
            rowp = ctx.enter_context(tc.tile_pool(name="rowp", bufs=2))
            scr = ctx.enter_context(tc.tile_pool(name="scr", bufs=2))
            recp = ctx.enter_context(tc.tile_pool(name="recp", bufs=3))
            spool = ctx.enter_context(tc.tile_pool(name="spool", bufs=1))
            ppool = ctx.enter_context(tc.tile_pool(name="ppool", bufs=2, space="PSUM"))
            pm128 = ctx.enter_context(tc.tile_pool(name="pm128", bufs=2, space="PSUM"))
            pm256 = ctx.enter_context(tc.tile_pool(name="pm256", bufs=2, space="PSUM"))
            ptp = ctx.enter_context(tc.tile_pool(name="ptp", bufs=2, space="PSUM"))

            # ---- constants ----
            id_bf = singles.tile([128, 128], bf16)
            make_identity(nc, id_bf)
            id_f32 = singles.tile([128, 128], fp32)
            make_identity(nc, id_f32)
            ident2 = id_f32  # for Horner (I - N Z)

            def tri_const(fillval, flip, nm):
                # value = p-f (flip=False) or f-p+1 (flip=True); keep 0 where
                # value > 0, else fill.  is_gt is the only op walrus codegens.
                t = singles.tile([128, 128], fp32, tag=nm, name=nm)
                nc.gpsimd.memset(t, 0.0)
                nc.gpsimd.affine_select(
                    out=t, in_=t, compare_op=OP.is_gt, fill=fillval,
                    base=(1 if flip else 0),
                    pattern=[[1 if flip else -1, 128]],
                    channel_multiplier=(-1 if flip else 1))
                return t

            LG_UPincl = tri_const(1e9, False, "lgup")    # 1e9 where p <= f
            LG_LOWstrict = tri_const(1e9, True, "lglo")  # 1e9 where p > f
            UT_incl = tri_const(1.0, False, "utinc")     # ones where p <= f
            LT_strict = tri_const(1.0, True, "ltst")     # ones where p > f
            ones_col = singles.tile([128, 1], bf16)
            nc.vector.memset(ones_col, 1.0)
            ones_row = singles.tile([1, 128], fp32)
            nc.vector.memset(ones_row, 1.0)
            l2eps = singles.tile([1, 1], fp32)
            nc.vector.memset(l2eps, 1e-6)

            from concourse.tile_rust import add_dep_helper as _adh

            def _collapse(insts):
                # SP nop chain: each nop syncs a few DMA/engine procs so no
                # later instruction (nor the strict barrier) needs more than
                # a couple of waits at once.
                n_ = nc.sync.nop()
                for i_ in insts:
                    _adh(n_.ins, i_.ins, True, "setup collapse")
                return n_

            cw_sb = singles.tile([128, 14, CONV], fp32)
            d1 = nc.sync.dma_start(out=cw_sb, in_=cw[:])
            scal_sb = singles.tile([128, 4], fp32)
            d2 = nc.sync.dma_start(out=scal_sb, in_=scal[:])
            _collapse([d1, d2])

            wqk_sb = wpool.tile([128, KT, 1024], bf16)
            d3 = nc.sync.dma_start(out=wqk_sb, in_=wqk_r)
            _collapse([d3])
            wv_sb = wpool.tile([128, KT, 768], bf16)
            d4 = nc.sync.dma_start(out=wv_sb, in_=wv_r)
            _collapse([d4])
            wba_sb = wpool.tile([128, KT, 66], bf16)
            d5 = nc.sync.dma_start(out=wba_sb, in_=wba_r)
            _collapse([d5])

            # persistent state per stream [dk=128p x (dk-tile, dv)] fp32
            S_sb = [spool.tile([128, 512], fp32, tag=f"S{s}", name=f"S{s}")
                    for s in range(3)]
            for s in range(3):
                nc.vector.memset(S_sb[s], 0.0)

            # collapse setup-phase deps (consts, weights, masks) to one sync
            tc.strict_bb_all_engine_barrier()

            # staging: data cols 4..515, halo cols 1..3 (prev block's tail)
            halo = {}

            def conv_silu(pp, inst, blk, out_tile):
                """pp: PSUM [128,512] raw proj; returns kappa bf16 [128,512]."""
                stg = stgp.tile([128, 516], fp32, tag="stg", name=f"stg{inst}_{blk}")
                nc.vector.tensor_copy(out=stg[:, 4:516], in_=pp[:, :])
                if blk == 0:
                    nc.vector.memset(stg[:, 0:4], 0.0)
                else:
                    nc.vector.tensor_copy(out=stg[:, 1:4], in_=halo[inst])
                if blk + 1 < nblk:
                    h_ = stgp.tile([128, 3], fp32, tag=f"halo{inst}",
                                   name=f"halo{inst}_{blk}")
                    nc.vector.tensor_copy(out=h_, in_=stg[:, 513:516])
                    halo[inst] = h_
                w = cw_sb[:, inst, :]
                acc = scr.tile([128, 512], fp32, tag="convacc")
                nc.vector.tensor_scalar_mul(acc, stg[:, 4:516], w[:, 3:4])
                nc.vector.scalar_tensor_tensor(
                    out=acc, in0=stg[:, 3:515], scalar=w[:, 2:3], in1=acc,
                    op0=OP.mult, op1=OP.add)
                nc.vector.scalar_tensor_tensor(
                    out=acc, in0=stg[:, 2:514], scalar=w[:, 1:2], in1=acc,
                    op0=OP.mult, op1=OP.add)
                nc.vector.scalar_tensor_tensor(
                    out=acc, in0=stg[:, 1:513], scalar=w[:, 0:1], in1=acc,
                    op0=OP.mult, op1=OP.add)
                sig = scr.tile([128, 512], bf16, tag="sig", name=f"sig{inst}_{blk}")
                nc.scalar.activation(out=sig, in_=acc, func=AF.Sigmoid)
                nc.vector.tensor_tensor(out=out_tile, in0=acc, in1=sig, op=OP.mult)
                return out_tile

            def project(lhs_sb, ncol_off, xtb, psum_tile, m=128):
                for kt in range(KT):
                    nc.tensor.matmul(
                        psum_tile[:m, :],
                        lhsT=lhs_sb[:, kt, ncol_off:ncol_off + m],
                        rhs=xtb[:, kt, :],
                        start=(kt == 0), stop=(kt == KT - 1))

            for blk in range(nblk):
                tsl = slice(blk * TB, (blk + 1) * TB)
                xtb = xpool.tile([128, KT, TB], bf16)
                for g in range(4):
                    nc.sync.dma_start(
                        out=xtb[:, 4 * g:4 * (g + 1), :],
                        in_=xT_r[:, 4 * g:4 * (g + 1), tsl])

                kap_q = {}
                kap_k = {}
                kap_v = {}
                skq = {}
                basg = {}
                for pair in range(2):
                    # --- q/k projections + conv + silu ---
                    for proj, kap in ((0, kap_q), (1, kap_k)):
                        for d in range(2):
                            pp = ppool.tile([128, TB], fp32, tag="pp")
                            project(wqk_sb, pair * 512 + proj * 256 + d * 128,
                                    xtb, pp)
                            kt_ = kapp.tile([128, TB], bf16,
                                            tag=f"kap{pair}_{proj}_{d}")
                            conv_silu(pp, cwi_qk(pair, proj, d), blk, kt_)
                            kap[(pair, d)] = kt_
                    # --- ba projection ---
                    pb = ppool.tile([33, TB], fp32, tag="pp", name="ppba")
                    for kt in range(KT):
                        nc.tensor.matmul(
                            pb[:, :], lhsT=wba_sb[:, kt, pair * 33:(pair + 1) * 33],
                            rhs=xtb[:, kt, :], start=(kt == 0), stop=(kt == KT - 1))
                    bg = rowp.tile([33, TB], fp32, tag=f"basg{pair}")
                    nc.vector.memset(bg, 0.0)
                    nc.scalar.activation(out=bg[0:1, :], in_=pb[0:1, :], func=AF.Sigmoid)
                    nc.scalar.activation(out=bg[32:33, :], in_=pb[32:33, :],
                                         func=AF.Sigmoid, scale=-1.0,
                                         bias=scal_sb[32:33, 2 * pair:2 * pair + 1])
                    nc.scalar.activation(out=bg[32:33, :], in_=bg[32:33, :],
                                         func=AF.Ln)
                    nc.vector.tensor_scalar_mul(bg[32:33, :], bg[32:33, :],
                                                scal_sb[32:33, 2 * pair + 1:2 * pair + 2])
                    basg[pair] = bg
                    # --- l2norm scale rows: sk@p0, sq@p32 ---
                    sk_t = rowp.tile([33, TB], fp32, tag=f"skq{pair}")
                    nc.vector.memset(sk_t, 0.0)
                    for proj, kap, prow in ((1, kap_k, 0), (0, kap_q, 32)):
                        ps = ptp.tile([1, TB], fp32, tag="tp", name="pssq")
                        for d in range(2):
                            sqr = scr.tile([128, TB], bf16, tag="sqr")
                            nc.scalar.activation(out=sqr, in_=kap[(pair, d)],
                                                 func=AF.Square)
                            nc.tensor.matmul(ps[:, :], lhsT=ones_col, rhs=sqr,
                                             start=(d == 0), stop=(d == 1))
                        row = sk_t[prow:prow + 1, :]
                        nc.scalar.activation(out=row, in_=ps[:, :], func=AF.Sqrt,
                                             bias=l2eps[0:1, :])
                        nc.vector.reciprocal(row, row)
                        if proj == 0:
                            nc.vector.tensor_scalar_mul(row, row, float(DK ** -0.5))
                    skq[pair] = sk_t

                for stream in range(3):
                    pair = 0 if stream < 2 else 1
                    # v projection for this stream
                    for d in range(2):
                        pp = ppool.tile([128, TB], fp32, tag="pp")
                        project(wv_sb, stream * 256 + d * 128, xtb, pp)
                        vt_ = kapp.tile([128, TB], bf16, tag=f"kapv{stream}_{d}")
                        conv_silu(pp, cwi_v(stream, d), blk, vt_)
                        kap_v[(stream, d)] = vt_

                for cc in range(NCH):
                    csl = slice(cc * C, (cc + 1) * C)
                    for stream in range(3):
                        pair = 0 if stream < 2 else 1
                        kq = [kap_q[(pair, d)][:, csl] for d in range(2)]
                        kk = [kap_k[(pair, d)][:, csl] for d in range(2)]
                        kv = [kap_v[(stream, d)][:, csl] for d in range(2)]
                        bg = basg[pair][:, csl]
                        sk_sl = skq[pair][:, csl]
                        S = S_sb[stream]

                        # -- transposes of scalar-row bundles -> column vectors
                        tp1 = ptp.tile([128, 33], fp32, tag="tp", name="tp1")
                        nc.tensor.transpose(tp1[:, :], bg, id_f32[0:33, 0:33])
                        cols_bg = recp.tile([128, 33], fp32, tag="cbg")
                        nc.vector.tensor_copy(out=cols_bg, in_=tp1)
                        tp2 = ptp.tile([128, 33], fp32, tag="tp", name="tp2")
                        nc.tensor.transpose(tp2[:, :], sk_sl, id_f32[0:33, 0:33])
                        cols_sk = recp.tile([128, 33], fp32, tag="csk")
                        nc.vector.tensor_copy(out=cols_sk, in_=tp2)
                        bcol = cols_bg[:, 0:1]
                        gcol = cols_bg[:, 32:33]
                        skcol = cols_sk[:, 0:1]
                        sqcol = cols_sk[:, 32:33]

                        # -- G columns: Gcol = UT_incl @ gcol ; GEcol = LT_strict @ gcol
                        pg = ptp.tile([128, 2], fp32, tag="tp", name="pg")
                        nc.tensor.matmul(pg[:, 0:1], lhsT=UT_incl, rhs=gcol,
                                         start=True, stop=True)
                        nc.tensor.matmul(pg[:, 1:2], lhsT=LT_strict, rhs=gcol,
                                         start=True, stop=True)
                        gc2 = recp.tile([128, 2], fp32, tag="gc2")
                        nc.vector.tensor_copy(out=gc2, in_=pg)
                        Gcol = gc2[:, 0:1]
                        GEcol = gc2[:, 1:2]

                        # derived columns: scl[:,0]=beta*sk, scl[:,1]=A=exp(G),
                        # scl[:,2]=-A*sk, scl[:,3]=A*sq, scl[:,4]=wU=sk*exp(GE),
                        # scl[:,5]=Aend=exp(G+GE)
                        scl = recp.tile([128, 6], fp32, tag="scl")
                        nc.vector.tensor_tensor(out=scl[:, 0:1], in0=bcol,
                                                in1=skcol, op=OP.mult)
                        nc.scalar.activation(out=scl[:, 1:2], in_=Gcol, func=AF.Exp)
                        nc.vector.scalar_tensor_tensor(
                            out=scl[:, 2:3], in0=scl[:, 1:2], scalar=-1.0,
                            in1=skcol, op0=OP.mult, op1=OP.mult)
                        nc.vector.tensor_scalar_mul(scl[:, 3:4], scl[:, 1:2], sqcol)
                        nc.scalar.activation(out=scl[:, 4:5], in_=GEcol, func=AF.Exp)
                        nc.vector.tensor_scalar_mul(scl[:, 4:5], scl[:, 4:5], skcol)
                        nc.vector.tensor_tensor(out=scl[:, 5:6], in0=Gcol,
                                                in1=GEcol, op=OP.add)
                        nc.scalar.activation(out=scl[:, 5:6], in_=scl[:, 5:6],
                                             func=AF.Exp)

                        # -- G row broadcast [128,128] via transpose + dma bcast
                        tg = ptp.tile([1, 128], fp32, tag="tp", name="tg")
                        nc.tensor.transpose(tg[:, :], Gcol, id_f32[:, :])
                        grow_row = recp.tile([1, 128], fp32, tag="growr")
                        nc.vector.tensor_copy(out=grow_row, in_=tg)
                        Grow = pm128.tile([128, 128], fp32, tag="m128", name="Grow")
                        nc.tensor.matmul(Grow[:, :], lhsT=ones_row, rhs=grow_row,
                                         start=True, stop=True)

                        # -- decay matrices
                        # D_M[t,s] = exp(G_t - G_s) for t>s else 0:
                        #   EM = (Grow - Gcol) + LG_UPincl ; D_M = exp(-EM)
                        em = scr.tile([128, 128], fp32, tag="em")
                        nc.vector.scalar_tensor_tensor(
                            out=em, in0=Grow, scalar=Gcol, in1=LG_UPincl,
                            op0=OP.subtract, op1=OP.add)
                        dm = recp.tile([128, 128], fp32, tag="dm")
                        nc.scalar.activation(out=dm, in_=em, func=AF.Exp, scale=-1.0)
                        # fold sk_s column factor into dm: dm *= SKrow
                        skrow_ps = pm128.tile([128, 128], fp32, tag="m128", name="skrow")
                        nc.tensor.matmul(skrow_ps[:, :], lhsT=ones_row,
                                         rhs=sk_sl[0:1, :], start=True, stop=True)
                        nc.vector.tensor_tensor(out=dm, in0=dm, in1=skrow_ps,
                                                op=OP.mult)
                        # D_AT[s,t] = exp(G_t - G_s) for s<=t else 0:
                        #   EA = (Grow - Gcol) - LG_LOWstrict ; D_AT = exp(EA)
                        ea = scr.tile([128, 128], fp32, tag="ea")
                        nc.vector.scalar_tensor_tensor(
                            out=ea, in0=Grow, scalar=Gcol, in1=LG_LOWstrict,
                            op0=OP.subtract, op1=OP.subtract)
                        dat = recp.tile([128, 128], fp32, tag="dat")
                        nc.scalar.activation(out=dat, in_=ea, func=AF.Exp)

                        # -- P = K K^T [t,s];  PqT = K Q^T [s,t]
                        pP = pm128.tile([128, 128], fp32, tag="m128", name="pP")
                        for d in range(2):
                            nc.tensor.matmul(pP[:, :], lhsT=kk[d], rhs=kk[d],
                                             start=(d == 0), stop=(d == 1))
                        pPq = pm128.tile([128, 128], fp32, tag="m128", name="pPq")
                        for d in range(2):
                            nc.tensor.matmul(pPq[:, :], lhsT=kk[d], rhs=kq[d],
                                             start=(d == 0), stop=(d == 1))
                        # M = (P * beta*sk_row(t)) * D_M   -> bf16
                        M = recp.tile([128, 128], bf16, tag="M")
                        nc.vector.scalar_tensor_tensor(
                            out=M, in0=pP, scalar=scl[:, 0:1], in1=dm,
                            op0=OP.mult, op1=OP.mult)
                        # AttnT = (PqT * sk_col(s)) * D_AT -> bf16
                        AT = recp.tile([128, 128], bf16, tag="AT")
                        nc.vector.scalar_tensor_tensor(
                            out=AT, in0=pPq, scalar=skcol, in1=dat,
                            op0=OP.mult, op1=OP.mult)

                        # -- Horner: Z = I; repeat k: Z = I - M^T Z
                        Z = id_bf
                        for it in range(k_horner):
                            nz = pm128.tile([128, 128], fp32, tag="m128", name="nz")
                            nc.tensor.matmul(nz[:, :], lhsT=M, rhs=Z,
                                             start=True, stop=True)
                            Zn = recp.tile([128, 128], bf16, tag="Zi")
                            nc.vector.scalar_tensor_tensor(
                                out=Zn, in0=nz, scalar=-1.0, in1=ident2,
                                op0=OP.mult, op1=OP.add)
                            Z = Zn

                        # -- S bf16 copy
                        Sb = recp.tile([128, 512], bf16, tag="Sb")
                        nc.vector.tensor_copy(out=Sb[:, 0:256], in_=S[:, 0:256])
                        nc.scalar.copy(out=Sb[:, 256:512], in_=S[:, 256:512])

                        # -- v transpose -> V [t, dv]
                        V = recp.tile([128, 256], bf16, tag="V")
                        for d in range(2):
                            tv = ptp.tile([128, 128], bf16, tag="tp", name="tv")
                            nc.tensor.transpose(tv[:, :], kv[d], id_bf)
                            nc.vector.tensor_copy(out=V[:, 128 * d:128 * (d + 1)],
                                                  in_=tv)

                        # -- KS, QS
                        pKS = pm256.tile([128, 256], fp32, tag="m256", name="pKS")
                        for d in range(2):
                            nc.tensor.matmul(pKS[:, :], lhsT=kk[d],
                                             rhs=Sb[:, 256 * d:256 * (d + 1)],
                                             start=(d == 0), stop=(d == 1))
                        pQS = pm256.tile([128, 256], fp32, tag="m256", name="pQS")
                        for d in range(2):
                            nc.tensor.matmul(pQS[:, :], lhsT=kq[d],
                                             rhs=Sb[:, 256 * d:256 * (d + 1)],
                                             start=(d == 0), stop=(d == 1))

                        # -- BR = beta * (V - A*sk*KS)
                        rt = scr.tile([128, 256], fp32, tag="rt")
                        nc.vector.scalar_tensor_tensor(
                            out=rt, in0=pKS, scalar=scl[:, 2:3], in1=V,
                            op0=OP.mult, op1=OP.add)
                        BR = recp.tile([128, 256], bf16, tag="BR")
                        nc.vector.tensor_scalar_mul(BR, rt, bcol)

                        # -- U = Z^T BR
                        pU = pm256.tile([128, 256], fp32, tag="m256", name="pU")
                        nc.tensor.matmul(pU[:, :], lhsT=Z, rhs=BR,
                                         start=True, stop=True)
                        Usb = recp.tile([128, 256], bf16, tag="Usb")
                        nc.vector.tensor_copy(out=Usb, in_=pU)
                        Upp = recp.tile([128, 256], bf16, tag="Upp")
                        nc.vector.tensor_scalar_mul(Upp, pU, scl[:, 4:5])

                        # -- o = sq * (A * QS + AttnT^T U)
                        pO2 = pm256.tile([128, 256], fp32, tag="m256", name="pO2")
                        nc.tensor.matmul(pO2[:, :], lhsT=AT, rhs=Usb,
                                         start=True, stop=True)
                        o2s = scr.tile([128, 256], fp32, tag="o2s")
                        nc.vector.tensor_scalar_mul(o2s, pO2, sqcol)
                        o_sb = recp.tile([128, 256], bf16, tag="osb")
                        nc.vector.scalar_tensor_tensor(
                            out=o_sb, in0=pQS, scalar=scl[:, 3:4], in1=o2s,
                            op0=OP.mult, op1=OP.add)
                        nc.sync.dma_start(
                            out=o_out[stream, blk * TB + cc * C: blk * TB + (cc + 1) * C, :],
                            in_=o_sb)

                        # -- kappa_k transpose [t, dk]
                        kkT = recp.tile([128, 256], bf16, tag="kkT")
                        for d in range(2):
                            tk = ptp.tile([128, 128], bf16, tag="tp", name="tk")
                            nc.tensor.transpose(tk[:, :], kk[d], id_bf)
                            nc.vector.tensor_copy(out=kkT[:, 128 * d:128 * (d + 1)],
                                                  in_=tk)

                        # -- state: S = Aend*S + K^T (wU*U)
                        pst = pm256.tile([128, 256], fp32, tag="m256", name="pst0")
                        nc.tensor.matmul(pst[:, :], lhsT=kkT[:, 0:128], rhs=Upp,
                                         start=True, stop=True)
                        pst1 = pm256.tile([128, 256], fp32, tag="m256", name="pst1")
                        nc.tensor.matmul(pst1[:, :], lhsT=kkT[:, 128:256], rhs=Upp,
                                         start=True, stop=True)
                        nc.vector.scalar_tensor_tensor(
                            out=S[:, 0:256], in0=S[:, 0:256], scalar=scl[:, 5:6],
                            in1=pst, op0=OP.mult, op1=OP.add)
                        nc.vector.scalar_tensor_tensor(
                            out=S[:, 256:512], in0=S[:, 256:512], scalar=scl[:, 5:6],
                            in1=pst1, op0=OP.mult, op1=OP.add)
    _split_multi_waits(nc)
    return nc


# ---------------------------------------------------------------------------
# host-side packing for launch A
# ---------------------------------------------------------------------------

def _stream_map():
    """returns per-core list of (b, h, j) for streams 0..2 and pair heads."""
    cores = []
    for c_ in range(N_CORES):
        b_ = c_ // 4
        cb = c_ % 4
        hA = cb
        hB = 4 + cb // 2
        jB = cb % 2
        cores.append(dict(b=b_, hA=hA, hB=hB,
                          streams=[(b_, hA, 0), (b_, hA, 1), (b_, hB, jB)]))
    return cores


def _prep_launch_a(ins, xT_bf):
    cores = _stream_map()
    Wq, Wk, Wv = ins["Wq"], ins["Wk"], ins["Wv"]
    Wb, Wa = ins["Wb"], ins["Wa"]
    cq, ck, cv = ins["conv_wq"], ins["conv_wk"], ins["conv_wv"]
    A_log, dt_bias = ins["A_log"], ins["dt_bias"]
    in_maps = []
    for cinfo in cores:
        b_, hA, hB = cinfo["b"], cinfo["hA"], cinfo["hB"]
        wqk = np.empty((HID, 1024), np.float32)
        wba = np.zeros((HID, 66), np.float32)
        cw = np.zeros((128, 14, CONV), np.float32)
        scalv = np.zeros((128, 4), np.float32)
        for p_, hh in ((0, hA), (1, hB)):
            wqk[:, p_ * 512:p_ * 512 + 256] = Wq[hh * DK:(hh + 1) * DK].T
            wqk[:, p_ * 512 + 256:p_ * 512 + 512] = Wk[hh * DK:(hh + 1) * DK].T
            wba[:, p_ * 33 + 0] = Wb[hh]
            wba[:, p_ * 33 + 32] = Wa[hh]
            for d in range(2):
                cw[:, p_ * 4 + 0 * 2 + d, :] = cq[hh * DK + d * 128: hh * DK + (d + 1) * 128]
                cw[:, p_ * 4 + 1 * 2 + d, :] = ck[hh * DK + d * 128: hh * DK + (d + 1) * 128]
            scalv[:, 2 * p_] = -dt_bias[hh]
            scalv[:, 2 * p_ + 1] = np.exp(A_log[hh])
        wv = np.empty((HID, 768), np.float32)
        for s_, (bb, hh, jj) in enumerate(cinfo["streams"]):
            off = hh * DV + jj * 256
            wv[:, s_ * 256:(s_ + 1) * 256] = Wv[off:off + 256].T
            for d in range(2):
                cw[:, 8 + s_ * 2 + d, :] = cv[off + d * 128: off + (d + 1) * 128]
        in_maps.append(dict(
            xT=xT_bf[b_], wqk=_bf16(wqk), wv=_bf16(wv), wba=_bf16(wba),
            cw=cw, scal=scalv))
    return in_maps


def kernel(**inputs):
    raise NotImplementedError("wired up after launch B")


# ---------------------------------------------------------------------------
# Launch B: RMSNorm + SiLU gate + Wo projection, token-sharded
# ---------------------------------------------------------------------------

def build_launch_b(ntok=B * T // N_CORES):
    import concourse.bass as bass
    import concourse.mybir as mybir
    import concourse.tile as tile
    from concourse.masks import make_identity

    fp32 = mybir.dt.float32
    bf16 = mybir.dt.bfloat16
    AF = mybir.ActivationFunctionType
    OP = mybir.AluOpType
    GD = H * DV          # 3072
    NT = ntok // 128     # token tiles
    NGT = GD // 128      # 24

    nc = bass.Bass()
    o_in = nc.declare_dram_parameter("o_in", [ntok, GD], bf16, isOutput=False)
    xT_tok = nc.declare_dram_parameter("xT_tok", [HID, ntok], bf16, isOutput=False)
    wgT = nc.declare_dram_parameter("wgT", [HID, GD], bf16, isOutput=False)
    woT = nc.declare_dram_parameter("woT", [GD, HID], bf16, isOutput=False)
    normw = nc.declare_dram_parameter("normw", [128, DV], fp32, isOutput=False)
    y = nc.declare_dram_parameter("y", [ntok, HID], fp32, isOutput=True)

    xT_r = xT_tok.rearrange("(kt p) t -> p kt t", p=128)
    wgT_r = wgT.rearrange("(kt p) n -> p kt n", p=128)
    woT_r = woT.rearrange("(gt p) n -> p gt n", p=128)
    o_r = o_in.rearrange("(nt p) g -> p nt g", p=128)

    with tile.TileContext(nc) as tc:
        import contextlib
        ctx = contextlib.ExitStack()
        with ctx:
            singles = ctx.enter_context(tc.tile_pool(name="singles", bufs=1))
            wsl = ctx.enter_context(tc.tile_pool(name="wsl", bufs=2))
            big = ctx.enter_context(tc.tile_pool(name="big", bufs=1))
            scr = ctx.enter_context(tc.tile_pool(name="scr", bufs=3))
            statp = ctx.enter_context(tc.tile_pool(name="statp", bufs=4))
            pgate = ctx.enter_context(tc.tile_pool(name="pgate", bufs=2, space="PSUM"))
            pout = ctx.enter_context(tc.tile_pool(name="pout", bufs=2, space="PSUM"))
            ptp = ctx.enter_context(tc.tile_pool(name="ptp", bufs=4, space="PSUM"))

            id_bf = singles.tile([128, 128], bf16)
            make_identity(nc, id_bf)
            nw_sb = singles.tile([128, DV], fp32)
            nc.sync.dma_start(out=nw_sb, in_=normw[:])
            epsb = singles.tile([128, 1], fp32)
            nc.vector.memset(epsb, NORM_EPS)

            xt_sb = big.tile([128, KT, ntok], bf16)
            for g in range(4):
                nc.sync.dma_start(out=xt_sb[:, 4 * g:4 * (g + 1), :],
                                  in_=xT_r[:, 4 * g:4 * (g + 1), :])
            # gated activations, transposed: [g-part, gtile, tok]
            actT = big.tile([128, NGT, ntok], bf16)

            # ---- phase 1: per (head, token-tile): stats, gate, mult, transpose
            for h_ in range(H):
                nsl = slice(h_ * 512, (h_ + 1) * 512)
                wg_s = wsl.tile([128, KT, 512], bf16, tag="wg", name=f"wg{h_}")
                for g in range(4):
                    nc.sync.dma_start(out=wg_s[:, 4 * g:4 * (g + 1), :],
                                      in_=wgT_r[:, 4 * g:4 * (g + 1), nsl])
                for t_ in range(NT):
                    tok = slice(t_ * 128, (t_ + 1) * 128)
                    ot = scr.tile([128, 512], bf16, tag="ot", name=f"ot{h_}_{t_}")
                    nc.sync.dma_start(out=ot, in_=o_r[:, t_, nsl])
                    st = statp.tile([128, 6], fp32, tag="bnst", name=f"st{h_}_{t_}")
                    nc.vector.bn_stats(out=st, in_=ot)
                    mv = statp.tile([128, 2], fp32, tag="bnmv", name=f"mv{h_}_{t_}")
                    nc.vector.bn_aggr(out=mv, in_=st)
                    m2 = statp.tile([128, 1], fp32, tag="m2", name=f"m2{h_}_{t_}")
                    nc.scalar.activation(out=m2, in_=mv[:, 0:1], func=AF.Square)
                    nc.vector.tensor_tensor(out=m2, in0=m2, in1=mv[:, 1:2], op=OP.add)
                    nc.scalar.activation(out=m2, in_=m2, func=AF.Sqrt,
                                         bias=epsb[:, 0:1])
                    rstd = statp.tile([128, 1], fp32, tag="rstd", name=f"rs{h_}_{t_}")
                    nc.vector.reciprocal(rstd, m2)

                    pg = pgate.tile([128, 512], fp32, tag="pg", name=f"pg{h_}_{t_}")
                    for kt in range(KT):
                        nc.tensor.matmul(pg[:, :], lhsT=xt_sb[:, kt, tok],
                                         rhs=wg_s[:, kt, :],
                                         start=(kt == 0), stop=(kt == KT - 1))
                    sig = scr.tile([128, 512], bf16, tag="sig", name=f"sg{h_}_{t_}")
                    nc.scalar.activation(out=sig, in_=pg, func=AF.Sigmoid)
                    gate = scr.tile([128, 512], fp32, tag="gate", name=f"gt{h_}_{t_}")
                    nc.vector.tensor_tensor(out=gate, in0=pg, in1=sig, op=OP.mult)
                    t1 = scr.tile([128, 512], fp32, tag="t1", name=f"t1{h_}_{t_}")
                    nc.vector.scalar_tensor_tensor(
                        out=t1, in0=ot, scalar=rstd, in1=gate,
                        op0=OP.mult, op1=OP.mult)
                    act = scr.tile([128, 512], bf16, tag="act", name=f"ac{h_}_{t_}")
                    nc.vector.tensor_tensor(out=act, in0=t1, in1=nw_sb, op=OP.mult)
                    for g4 in range(4):
                        gt = h_ * 4 + g4
                        tp = ptp.tile([128, 128], bf16, tag="tp",
                                      name=f"tp{h_}_{t_}_{g4}")
                        nc.tensor.transpose(tp[:, :],
                                            act[:, g4 * 128:(g4 + 1) * 128], id_bf)
                        nc.vector.tensor_copy(out=actT[:, gt, tok], in_=tp)

            # ---- phase 2: y = act @ Wo^T ----
            for n_ in range(HID // 512):
                nsl = slice(n_ * 512, (n_ + 1) * 512)
                wo_s = wsl.tile([128, NGT, 512], bf16, tag="wo", name=f"wo{n_}")
                for g in range(4):
                    nc.sync.dma_start(
                        out=wo_s[:, 6 * g:6 * (g + 1), :],
                        in_=woT_r[:, 6 * g:6 * (g + 1), nsl])
                for t_ in range(NT):
                    tok = slice(t_ * 128, (t_ + 1) * 128)
                    po = pout.tile([128, 512], fp32, tag="po", name=f"po{n_}_{t_}")
                    for gt in range(NGT):
                        nc.tensor.matmul(po[:, :], lhsT=actT[:, gt, tok],
                                         rhs=wo_s[:, gt, :],
                                         start=(gt == 0), stop=(gt == NGT - 1))
                    yo = scr.tile([128, 512], fp32, tag="yo", name=f"yo{n_}_{t_}")
                    nc.scalar.copy(out=yo, in_=po)
                    nc.sync.dma_start(out=y[tok, nsl], in_=yo)
    _split_multi_waits(nc)
    return nc


def _prep_launch_b(ins, xT_bf, o_full):
    """o_full: [B*T, 3072] bf16; xT_bf: per-batch [HID, T] bf16."""
    ntok = B * T // N_CORES
    wgT = _bf16(np.ascontiguousarray(ins["Wg"].T))
    woT = _bf16(np.ascontiguousarray(ins["Wo"].T))
    nw = np.broadcast_to(ins["norm_w"].astype(np.float32), (128, DV)).copy()
    in_maps = []
    for c_ in range(N_CORES):
        t0 = c_ * ntok
        b0_ = t0 // T
        xslice = np.ascontiguousarray(xT_bf[b0_][:, t0 - b0_ * T: t0 - b0_ * T + ntok])
        in_maps.append(dict(
            o_in=np.ascontiguousarray(o_full[t0:t0 + ntok]),
            xT_tok=xslice, wgT=wgT, woT=woT, normw=nw))
    return in_maps


# ---------------------------------------------------------------------------
# Full pipeline
# ---------------------------------------------------------------------------

_CACHE = {}


def kernel(**inputs):
    global LAST_HW_EXEC_NS
    from concourse.bass_utils import run_bass_kernel_spmd
    ins = {k: np.asarray(v) for k, v in inputs.items()}
    x = ins['hidden_states'].astype(np.float32)
    xT_bf = [np.ascontiguousarray(_bf16(x[b_]).T) for b_ in range(B)]

    in_maps_a = _prep_launch_a(ins, xT_bf)
    if 'a' not in _CACHE:
        _CACHE['a'] = build_launch_a()
    res_a = run_bass_kernel_spmd(_CACHE['a'], in_maps_a, list(range(N_CORES)))

    o_full = np.empty((B * T, H * DV), BF)
    for c_, cinfo in enumerate(_stream_map()):
        oc = np.asarray(res_a.results[c_]["o"])
        for s_, (b_, h_, j_) in enumerate(cinfo["streams"]):
            o_full[b_ * T:(b_ + 1) * T,
                   h_ * DV + j_ * 256: h_ * DV + (j_ + 1) * 256] = oc[s_]

    in_maps_b = _prep_launch_b(ins, xT_bf, o_full)
    if 'b' not in _CACHE:
        _CACHE['b'] = build_launch_b()
    res_b = run_bass_kernel_spmd(_CACHE['b'], in_maps_b, list(range(N_CORES)))
    y = np.concatenate([np.asarray(res_b.results[c_]["y"])
                        for c_ in range(N_CORES)], 0)

    ns = 0
    for r_ in (res_a, res_b):
        if getattr(r_, "exec_time_ns", None):
            ns += r_.exec_time_ns
    LAST_HW_EXEC_NS = ns if ns > 0 else None
    return np.ascontiguousarray(y.reshape(B, T, HID).astype(np.float32))

